# revision 5
# baseline (speedup 1.0000x reference)
"""CombinedAttentionProcessor kernel for 8 Trainium2 NeuronCores.

Problem: B=2, S=4096, C=640, H=8 heads, D=80 head_dim.
    q/k/v = hs @ W{q,k,v}.T ; per-(b,h): softmax(q k^T / sqrt(D)) v ;
    out = attn @ Wo.T + bo + residual.

Sharding: 16 (batch, head) groups -> 2 per core (batch-parallel over B,
head-parallel over H). Each core computes its 2 heads' full attention and a
partial output projection [S, C]; the host sums the 4 partials per batch and
adds bias + residual.

Per-core dataflow (all matmuls fp32r = full PE rate, probs/V in bf16):
  Phase A: load hsT [C, S]; project qT/kT [D, S] (d-major) and v [S, D]
           (natural, with a ones column appended for the softmax row-sum).
  Phase B: per head, per 512-query chunk: scoresT tiles [128 keys, 512 q]
           via PE; exp(scale*x) on ScalarE (PSUM->SBUF, bf16); AV matmul
           accumulates out_avT [81, 512] over the 32 key tiles -- row 80 is
           the softmax denominator (ones column of V). Normalize with a
           reciprocal + K=1-style broadcast matmul + DVE multiply.
  Phase C: output projection per 128-query tile: o = sum_h avT_h.T @ WoT_h,
           accumulated in PSUM over both heads, DMA'd out.

fp32r matmuls admit only ONE sync-wait; dummy matmuls right after the input
DMAs make PE observe every DMA-queue semaphore once, and engine assignment
keeps every real matmul's unobserved waits on a single semaphore.
"""
import sys

if "/opt/trn_rl_repo" not in sys.path:
    sys.path.insert(0, "/opt/trn_rl_repo")

import numpy as np

B, S, C = 2, 4096, 640
H, D = 8, 80
HPC = 2          # heads per core
NCORES = 8
KC = C // 128    # 5 contraction tiles over C
SCALE = 1.0 / float(np.sqrt(D))

_NC_CACHE = {}


def build_nc(s=S):
    import concourse.bacc as bacc
    import concourse.mybir as mybir
    import concourse.tile as tile
    from concourse.tile import add_dep_helper

    f32 = mybir.dt.float32
    f32r = mybir.dt.float32r
    bf16 = mybir.dt.bfloat16

    njt = s // 128   # key tiles
    nch = s // 512   # query chunks
    assert s % 1024 == 0

    nc = bacc.Bacc("TRN2", target_bir_lowering=False, debug=False,
                   num_devices=NCORES)

    hsT = nc.dram_tensor("hsT", [C, s], f32r, kind="ExternalInput")
    wq = nc.dram_tensor("wq", [C, HPC * D], f32r, kind="ExternalInput")
    wk = nc.dram_tensor("wk", [C, HPC * D], f32r, kind="ExternalInput")
    wv = nc.dram_tensor("wv", [C, HPC * D], f32r, kind="ExternalInput")
    wo = nc.dram_tensor("wo", [HPC * D, C], f32r, kind="ExternalInput")
    o_dram = nc.dram_tensor("o", [s, C], f32, kind="ExternalOutput")

    with tile.TileContext(nc) as tc:
        with (
            tc.tile_pool(name="persist", bufs=1) as pp,
        ):
            # ---- persistent tiles (live for the whole kernel) ----
            qT = [pp.tile([128, s], f32r, name=f"qT{h}") for h in range(HPC)]
            kT = [pp.tile([128, s], f32r, name=f"kT{h}") for h in range(HPC)]
            # v tiles per key-tile, per-head stride 97:
            # [head data (80) | zero pad (16) | one] -- the ones column sits
            # at partition-aligned position 96 so the AV matmul's row sum
            # lands on PSUM partition 96 (engine APs need 32-aligned bases)
            VS = 97
            v_sb = pp.tile([128, njt, 2 * VS], bf16, name="v_sb")
            wo_sb = pp.tile([128, HPC, C], f32r, name="wo_sb")

            # zero-init full tiles (engine APs can't start at partition 80,
            # so pad rows are zeroed whole-tile, then rows 0..79 get
            # overwritten). Memset can't write float32r, so f32r tiles are
            # filled via DVE tensor_copy from a broadcast f32 source (the
            # engine cast satisfies the verifier's fp32r rounding rule).
            # All on DVE so matmul waits merge on one semaphore.
            zsrc = pp.tile([128, 8], f32, name="zsrc")
            osrc = pp.tile([128, 8], f32, name="osrc")
            nc.vector.memset(zsrc[:], 0.0)
            nc.vector.memset(osrc[:], 1.0)

            def zfill(dst2d):
                nc.vector.tensor_copy(
                    dst2d, zsrc[:dst2d.shape[0], 0:1].broadcast_to(dst2d.shape))

            zfill(wo_sb.rearrange("p a b -> p (a b)"))
            nc.vector.memset(v_sb[:, :, :], 0.0)
            nc.vector.memset(v_sb[:, :, VS - 1], 1.0)
            nc.vector.memset(v_sb[:, :, 2 * VS - 1], 1.0)
            for h in range(HPC):
                zfill(kT[h][:, :])
                zfill(qT[h][:, :])

            # ================= Phase A: projections =================
            with (
                tc.tile_pool(name="pA", bufs=1) as pA,
                tc.tile_pool(name="pAqk_ps", bufs=2, space="PSUM") as pAqk,
                tc.tile_pool(name="pAv_ps", bufs=2, space="PSUM") as pAv,
                tc.tile_pool(name="pdum_ps", bufs=1, space="PSUM") as pdum,
            ):
                hsT_sb = pA.tile([128, KC, s], f32r, name="hsT_sb")
                wq_sb = pA.tile([128, KC, HPC * D], f32r, name="wq_sb")
                wk_sb = pA.tile([128, KC, HPC * D], f32r, name="wk_sb")
                wv_sb = pA.tile([128, KC, HPC * D], f32r, name="wv_sb")

                half = s // 2
                for kc in range(KC):
                    for ih in range(2):
                        nc.sync.dma_start(
                            hsT_sb[:, kc, ih * half:(ih + 1) * half],
                            hsT[kc * 128:(kc + 1) * 128,
                                ih * half:(ih + 1) * half],
                        )
                for w_sb, w_dr in ((wq_sb, wq), (wk_sb, wk), (wv_sb, wv)):
                    nc.sync.dma_start(
                        w_sb[:, :, :],
                        w_dr.ap().rearrange("(kc p) m -> p kc m", p=128),
                    )
                nc.sync.dma_start(
                    wo_sb[0:D, :, :],
                    wo.ap().rearrange("(h p) n -> p h n", p=D),
                )

                # dummy matmuls: make PE observe every DMA-queue semaphore
                # (fp32r matmuls can carry only one sync wait each); the
                # first one observes the DVE memsets so the wo dummy below
                # (whose region was memset AND DMA'd) has only the DMA wait
                dum = pdum.tile([8, 8], f32, name="dum")
                src = qT[0][0:1, 0:8]
                nc.tensor.matmul(dum[:], src, src, start=True, stop=True,
                                 skip_group_check=True)
                for kc in range(KC):
                    for ih in range(2):
                        src = hsT_sb[0:1, kc, ih * half:ih * half + 8]
                        nc.tensor.matmul(dum[:], src, src, start=True,
                                         stop=True, skip_group_check=True)
                for w_sb in (wq_sb, wk_sb, wv_sb):
                    src = w_sb[0:1, 0, 0:8]
                    nc.tensor.matmul(dum[:], src, src, start=True, stop=True,
                                     skip_group_check=True)
                src = wo_sb[0:1, 0, 0:8]
                nc.tensor.matmul(dum[:], src, src, start=True, stop=True,
                                 skip_group_check=True)

                # qT / kT projections: qT[d, i] = sum_c Wq[d, c] hsT[c, i]
                for h in range(HPC):
                    for tname, w_sb, dst in (("q", wq_sb, qT[h]),
                                             ("k", wk_sb, kT[h])):
                        for iq in range(nch):
                            ps = pAqk.tile([D, 512], f32, name="qk_ps")
                            for kc in range(KC):
                                nc.tensor.matmul(
                                    ps[:],
                                    w_sb[:, kc, h * D:(h + 1) * D],
                                    hsT_sb[:, kc, iq * 512:(iq + 1) * 512],
                                    start=(kc == 0), stop=(kc == KC - 1),
                                )
                            cp = nc.vector.tensor_copy(
                                dst[0:D, iq * 512:(iq + 1) * 512], ps[:])
                            del cp
                # v projection (both heads fused): v[j, d2] natural layout
                for jt in range(njt):
                    ps = pAv.tile([128, HPC * D], f32, name="v_ps")
                    for kc in range(KC):
                        nc.tensor.matmul(
                            ps[:],
                            hsT_sb[:, kc, jt * 128:(jt + 1) * 128],
                            wv_sb[:, kc, :],
                            start=(kc == 0), stop=(kc == KC - 1),
                        )
                    c0 = nc.vector.tensor_copy(v_sb[:, jt, 0:D], ps[:, 0:D])
                    c1 = nc.vector.tensor_copy(v_sb[:, jt, VS:VS + D],
                                               ps[:, D:2 * D])
                    del c0, c1

            # ================= Phase B + C =================
            with (
                tc.tile_pool(name="pB", bufs=1) as pB,
                tc.tile_pool(name="ppt", bufs=3) as ppt,
                tc.tile_pool(name="pbcs", bufs=2) as pbcs,
                tc.tile_pool(name="posb", bufs=2) as posb,
                tc.tile_pool(name="psc_ps", bufs=2, space="PSUM") as psc,
                tc.tile_pool(name="pav_ps", bufs=1, space="PSUM") as pav,
                tc.tile_pool(name="pbc_ps", bufs=1, space="PSUM") as pbc,
                tc.tile_pool(name="po_ps", bufs=1, space="PSUM") as pops,
            ):
                avn = [pB.tile([128, s], f32r, name=f"avn{h}")
                       for h in range(HPC)]
                recip_sb = pB.tile([128, 512], f32r, name="recip_sb")
                ones_sb = pB.tile([128, D], f32r, name="ones_sb")
                for h in range(HPC):
                    zfill(avn[h][:, :])
                zfill(recip_sb[:, :])
                zfill(ones_sb[:, :])
                nc.vector.tensor_copy(
                    ones_sb[0:1, :], osrc[0:1, 0:1].broadcast_to([1, D]))

                for i8 in range(nch):
                    i0 = i8 * 512
                    mul_h0 = None
                    for h in range(HPC):
                        if h == 1 and mul_h0 is not None:
                            # absorb the av-slot WAR (DVE) on a dummy so the
                            # first AV matmul below carries only the ACT wait
                            dum2 = pbc.tile([8, 8], f32, name="dum2",
                                            tag="bcslot")
                            dmm = nc.tensor.matmul(
                                dum2[:], ones_sb[0:1, 0:8], ones_sb[0:1, 0:8],
                                start=True, stop=True, skip_group_check=True)
                            add_dep_helper(dmm.ins, mul_h0.ins,
                                           reason="absorb av WAR on PE")
                        av = pav.tile([VS, 512], f32, name="av_ps")
                        for jg in range(njt // 2):
                            sc = psc.tile([128, 1024], f32, name="sc_ps")
                            for jj in range(2):
                                j = 2 * jg + jj
                                nc.tensor.matmul(
                                    sc[:, jj * 512:(jj + 1) * 512],
                                    kT[h][:, j * 128:(j + 1) * 128],
                                    qT[h][:, i0:i0 + 512],
                                    start=True, stop=True,
                                )
                            pt = ppt.tile([128, 1024], bf16, name="pt")
                            nc.scalar.activation(
                                out=pt[:], in_=sc[:],
                                func=mybir.ActivationFunctionType.Exp,
                                scale=SCALE,
                            )
                            for jj in range(2):
                                j = 2 * jg + jj
                                nc.tensor.matmul(
                                    av[:],
                                    v_sb[:, j, h * VS:(h + 1) * VS],
                                    pt[:, jj * 512:(jj + 1) * 512],
                                    start=(j == 0), stop=(j == njt - 1),
                                )
                        # normalize: avn = av[0:D] * (1 / rowsum) broadcast
                        with nc.allow_low_precision(
                                reason="fp32r recip feeds broadcast matmul"):
                            nc.vector.reciprocal(recip_sb[0:1, :],
                                                 av[VS - 1:VS, :])
                        bc = pbc.tile([D, 512], f32, name="bc_ps",
                                      tag="bcslot")
                        nc.tensor.matmul(bc[:], ones_sb[:], recip_sb[:],
                                         start=True, stop=True)
                        bc_sb = pbcs.tile([D, 512], f32, name="bc_sb")
                        nc.vector.tensor_copy(bc_sb[:], bc[:])
                        mul = nc.vector.tensor_mul(
                            avn[h][0:D, i0:i0 + 512], av[0:D, :], bc_sb[:])
                        if h == 0:
                            mul_h0 = mul

                    # Phase C for this 512-query chunk
                    for it in range(4):
                        t0 = i0 + it * 128
                        o_ps = pops.tile([128, C], f32, name="o_ps")
                        for n0, n1 in ((0, 512), (512, C)):
                            for h in range(HPC):
                                nc.tensor.matmul(
                                    o_ps[:, n0:n1],
                                    avn[h][:, t0:t0 + 128],
                                    wo_sb[:, h, n0:n1],
                                    start=(h == 0), stop=(h == HPC - 1),
                                )
                        o_sb = posb.tile([128, C], f32, name="o_sb")
                        nc.vector.tensor_copy(o_sb[:], o_ps[:])
                        nc.sync.dma_start(o_dram[t0:t0 + 128, :], o_sb[:])

    nc.compile()
    return nc


def _get_nc(s=S):
    if s not in _NC_CACHE:
        _NC_CACHE[s] = build_nc(s)
    return _NC_CACHE[s]


def make_in_maps(hidden_states, Wq, Wk, Wv, Wo):
    """Shard full inputs into 8 per-core input dicts."""
    hs = np.ascontiguousarray(np.asarray(hidden_states, dtype=np.float32))
    Wq = np.asarray(Wq, dtype=np.float32)
    Wk = np.asarray(Wk, dtype=np.float32)
    Wv = np.asarray(Wv, dtype=np.float32)
    Wo = np.asarray(Wo, dtype=np.float32)
    hsT = [np.ascontiguousarray(hs[b].T) for b in range(B)]
    in_maps = []
    for c in range(NCORES):
        b, hp = divmod(c, NCORES // B)
        rows = slice(HPC * D * hp, HPC * D * (hp + 1))
        in_maps.append({
            "hsT": hsT[b],
            "wq": np.ascontiguousarray(Wq[rows, :].T),
            "wk": np.ascontiguousarray(Wk[rows, :].T),
            "wv": np.ascontiguousarray(Wv[rows, :].T),
            "wo": np.ascontiguousarray(Wo[:, rows].T),
        })
    return in_maps


def assemble(results, hidden_states, bo):
    hs = np.asarray(hidden_states, dtype=np.float32)
    bo = np.asarray(bo, dtype=np.float32)
    out = np.empty((B, S, C), dtype=np.float32)
    ncb = NCORES // B
    for b in range(B):
        acc = results[b * ncb]["o"].astype(np.float64)
        for k in range(1, ncb):
            acc = acc + results[b * ncb + k]["o"]
        out[b] = (acc + bo[None, :]).astype(np.float32) + hs[b]
    return out


def kernel(hidden_states, Wq, Wk, Wv, Wo, bo):
    from concourse.bass_utils import run_bass_kernel_spmd

    nc = _get_nc(S)
    in_maps = make_in_maps(hidden_states, Wq, Wk, Wv, Wo)
    res = run_bass_kernel_spmd(nc, in_maps, core_ids=list(range(NCORES)))
    return assemble(res.results, hidden_states, bo)
